# revision 15
# baseline (speedup 1.0000x reference)
"""Multi-head attention (B=2, S=2048, D=1024, H=16) on 8 trn2 NeuronCores.

Sharding: core c = (b, g) with b = c // 4 (data parallel over batch) and
g = c % 4 (tensor parallel over heads, 4 heads per core).  Each core
computes q/k/v projections for its 4 heads, attention, and a partial
output projection (row-parallel Wo); the host sums the 4 partials per
batch and adds bo_eff = bo + Wo @ bv.

v3 schedule: the ACT engine (exp over the 16.8M scores per core) is the
hard bottleneck (~0.85ns/col + ~450ns per-ACTIVATE overhead), so the
kernel keeps ACT busy continuously from the earliest possible moment:

- exp instructions are N=1536 (3 PSUM banks x 2 buffers; 10x1536 + 1024
  per (sq-tile, head-pair) group), amortizing the ACTIVATE overhead.
- only K-proj(m0) + Q-proj(m0,s0) precede the first scores; every other
  projection (k m1, q s1-3, v) plus PV / normalize / out-proj is fed
  from a budgeted work queue pumped in the PE slack of each scores unit,
  so the in-order PE stream never blocks ahead of pending scores.
- inputs stage as single [128, 8, 2048] SBUF tiles (one DMA issue per
  512-col chunk) split over the sync/vector/gpsimd queues.
- PE is pre-warmed with dummy matmuls during the DMA head (HAM at full
  clock when the projections start).
- PSUM: 2x3-bank rotating scores slots + one 2-bank slot shared (in
  strict FIFO) by warmup, filler projections, v-proj, PV accumulation
  and interleaved out-projection.
- softmax denominators ride as a ones-column in v (row 64 of the PV
  accumulator); normalization divides attn rows; the reciprocal reads
  the sums row directly from PSUM; all PSUM->SBUF copies are on DVE.
"""

import contextlib
import sys

import numpy as np

for _p in ("/opt/trn_rl_repo", "/root/.axon_site/_ro/trn_rl_repo"):
    if _p not in sys.path:
        sys.path.insert(0, _p)

B, S, D = 2, 2048, 1024
H, DK = 16, 64
HPC = 4  # heads per core
HD = HPC * DK  # 256 head-dims per core
NCORES = 8
SCALE = 1.0 / 8.0  # 1/sqrt(DK)
LOG2E = float(np.log2(np.e))
LN2 = float(np.log(2.0))

KT = D // 128  # 8 contraction tiles for the projections
NS = S // 512  # 4 sq tiles
NB = S // 128  # 16 sk blocks (units) per group
EXPW = 1536  # exp instruction width (3 psum banks)
NE = 11  # exps per group: 10 x 1536 + 1 x 1024

_CACHE = {}


def _build_nc():
    from concourse import bacc
    import concourse.mybir as mybir
    import concourse.tile as tile

    F32 = mybir.dt.float32
    BF16 = mybir.dt.bfloat16
    Exp = mybir.ActivationFunctionType.Exp

    nc = bacc.Bacc(None)

    qt_d = nc.dram_tensor("qt", [D, S], BF16, kind="ExternalInput")
    kt_d = nc.dram_tensor("kt", [D, S], BF16, kind="ExternalInput")
    vt_d = nc.dram_tensor("vt", [D, S], BF16, kind="ExternalInput")
    wqt_d = nc.dram_tensor("wqt", [D, HD], BF16, kind="ExternalInput")
    wkt_d = nc.dram_tensor("wkt", [D, HD], BF16, kind="ExternalInput")
    wvt_d = nc.dram_tensor("wvt", [D, HD], BF16, kind="ExternalInput")
    wot_d = nc.dram_tensor("wot", [HD, D], BF16, kind="ExternalInput")
    bq_d = nc.dram_tensor("bq", [HD], F32, kind="ExternalInput")
    bk_d = nc.dram_tensor("bk", [HD], F32, kind="ExternalInput")
    out_d = nc.dram_tensor("out", [S, D], BF16, kind="ExternalOutput")

    with tile.TileContext(nc) as tc, contextlib.ExitStack() as ctx:
        consts = ctx.enter_context(tc.tile_pool(name="consts", bufs=1))
        wpool = ctx.enter_context(tc.tile_pool(name="wpool", bufs=4))
        xpool = ctx.enter_context(tc.tile_pool(name="xpool", bufs=5))
        persist = ctx.enter_context(tc.tile_pool(name="persist", bufs=1))
        probsp = ctx.enter_context(tc.tile_pool(name="probsp", bufs=20))
        smallp = ctx.enter_context(tc.tile_pool(name="smallp", bufs=1))
        outp = ctx.enter_context(tc.tile_pool(name="outp", bufs=2))
        psum = ctx.enter_context(tc.tile_pool(name="psum", bufs=2, space="PSUM"))

        # ---- constants / biases (sync: tiny, land first) ----
        bq_sb = consts.tile([128, 2], F32)  # col m = bq[128m : 128(m+1)]
        nc.sync.dma_start(out=bq_sb[:], in_=bq_d[:].rearrange("(m p) -> p m", p=128))
        bk_sb = consts.tile([128, 2], F32)
        nc.sync.dma_start(out=bk_sb[:], in_=bk_d[:].rearrange("(m p) -> p m", p=128))

        wk_sb = wpool.tile([128, KT, HD], BF16, tag="w", name="wk")
        nc.sync.dma_start(
            out=wk_sb[:], in_=wkt_d[:].rearrange("(kt p) m -> p kt m", p=128)
        )
        wq_sb = wpool.tile([128, KT, HD], BF16, tag="w", name="wq")
        nc.sync.dma_start(
            out=wq_sb[:], in_=wqt_d[:].rearrange("(kt p) m -> p kt m", p=128)
        )
        wv_sb = wpool.tile([128, KT, HD], BF16, tag="w", name="wv")
        nc.gpsimd.dma_start(
            out=wv_sb[:], in_=wvt_d[:].rearrange("(kt p) m -> p kt m", p=128)
        )

        ones_sb = consts.tile([128, HPC], F32)
        nc.vector.memset(ones_sb[:], 1.0)
        # load the ACT exp table immediately so the first real exp is fast
        warm = consts.tile([1, 2], F32)
        nc.scalar.activation(warm[:], ones_sb[0:1, 0:2], Exp, scale=LN2)

        # ---- input staging: half-S tiles [128, KT, 1024].  K chunks pace
        # the score stream (sync ring); Q s0 + V land early on the scalar
        # ring so vproj/PV trail the exp stream closely; the second V half
        # reuses K's first slot once the k-projection is done. ----
        def halfload(eng, dram, dst, h):
            eng.dma_start(
                out=dst[:, :, :],
                in_=dram[:, h * 1024 : (h + 1) * 1024].rearrange(
                    "(kt p) c -> p kt c", p=128
                ),
            )

        k_h = [xpool.tile([128, KT, 1024], BF16, tag="x", name=f"k_h{h}") for h in range(2)]
        for h in range(2):
            halfload(nc.sync, kt_d, k_h[h], h)
        q_h0 = xpool.tile([128, KT, 1024], BF16, tag="x", name="q_h0")
        halfload(nc.scalar, qt_d, q_h0, 0)
        v_h0 = xpool.tile([128, KT, 1024], BF16, tag="x", name="v_h0")
        halfload(nc.scalar, vt_d, v_h0, 0)
        q_h1 = xpool.tile([128, KT, 1024], BF16, tag="x", name="q_h1")
        halfload(nc.scalar, qt_d, q_h1, 1)
        v_h1 = xpool.tile([128, KT, 1024], BF16, tag="x", name="v_h1")
        halfload(nc.scalar, vt_d, v_h1, 1)
        q_h = [q_h0, q_h1]
        v_h = [v_h0, v_h1]
        wo_sb = wpool.tile([128, 2, D], BF16, tag="w", name="wo")  # [p][pair][dm]
        nc.gpsimd.dma_start(
            out=wo_sb[:], in_=wot_d[:].rearrange("(m p) n -> p m n", p=128)
        )

        # ---- persistent activations ----
        qT = [persist.tile([128, S], BF16, name=f"qT{m}") for m in range(2)]
        kT = [persist.tile([128, S], BF16, name=f"kT{m}") for m in range(2)]
        vplus = [
            persist.tile([128, HPC, DK + 1], BF16, name=f"vp{i}") for i in range(NB)
        ]
        for i in range(NB):
            nc.vector.tensor_copy(
                vplus[i][:, :, DK : DK + 1],
                ones_sb[:].rearrange("p (h o) -> p h o", o=1),
            )
        attnT = [persist.tile([128, S], BF16, name=f"attnT{p}") for p in range(2)]

        # ---- helpers ----
        def proj_group(w_sb, bias_sb, dst, x_halves, m, s, tag):
            ps = psum.tile(
                [128, 512], F32, tag=tag, bufs=(2 if tag == "sc" else 1),
                name=f"pj{m}{s}",
            )
            xh = x_halves[s // 2]
            col = (s % 2) * 512
            for k in range(KT):
                nc.tensor.matmul(
                    ps[:],
                    w_sb[:, k, m * 128 : (m + 1) * 128],
                    xh[:, k, col : col + 512],
                    start=(k == 0),
                    stop=(k == KT - 1),
                )
            nc.vector.tensor_scalar_add(
                dst[m][:, s * 512 : (s + 1) * 512], ps[:], bias_sb[:, m : m + 1]
            )

        vstate = {"ps": None}

        def vproj_one(sb):
            # all 16 blocks ping-pong through one 2-bank psum tile
            if vstate["ps"] is None:
                vstate["ps"] = psum.tile(
                    [128, 1024], F32, tag="acc", bufs=1, name="vprojps"
                )
            ps = vstate["ps"]
            col = (sb % 2) * 512
            vh = v_h[sb // 8]
            vcol = (sb % 8) * 128
            for k in range(KT):
                nc.tensor.matmul(
                    ps[:, col : col + HD],
                    vh[:, k, vcol : vcol + 128],
                    wv_sb[:, k, :],
                    start=(k == 0),
                    stop=(k == KT - 1),
                )
            nc.vector.tensor_copy(
                vplus[sb][:, :, 0:DK],
                ps[:, col : col + HD].rearrange("p (h d) -> p h d", h=HPC),
            )
            if sb == NB - 1:
                vstate["ps"] = None  # release slot for the pv chain

        def unit_map(u, j):
            # (unit, head) -> (exp index, col offset) within a group
            c = 1024 * u + 512 * j
            if c >= 15360:
                return 10, c - 15360
            return c // 1536, c % 1536

        def exp_done_unit(e):
            # unit at which exp e's last scores chunk is emitted
            if e == 10:
                return 15
            return (1536 * (e + 1) - 512) // 1024

        def emit_scores(g, u):
            t, p = g["t"], g["p"]
            tsl = slice(t * 512, (t + 1) * 512)
            done = []
            for j in range(2):
                e, off = unit_map(u, j)
                if g["sc"][e] is None:
                    w = EXPW if e < 10 else 1024
                    g["sc"][e] = psum.tile(
                        [128, w], F32, tag="sc", bufs=2, name=f"sc{t}{p}{e}"
                    )
                hsl = slice(64 * j, 64 * (j + 1))
                nc.tensor.matmul(
                    g["sc"][e][:, off : off + 512],
                    kT[p][hsl, u * 128 : (u + 1) * 128],
                    qT[p][hsl, tsl],
                    start=True,
                    stop=True,
                    tile_position=(64 * j, 0),
                )
                if (e < 10 and off == 1024) or (u, j) == (15, 1):
                    done.append(e)
            return done

        def emit_exp(g, e):
            w = EXPW if e < 10 else 1024
            probs = probsp.tile(
                [128, w], BF16, tag="probs", bufs=20, name=f"pr{g['t']}{g['p']}{e}"
            )
            nc.scalar.activation(probs[:], g["sc"][e][:], Exp, scale=LN2)
            g["probs"][e] = probs
            g["sc"][e] = None

        def alloc_acc(g):
            g["acc"] = psum.tile(
                [DK + 1, 1024], F32, tag="acc", bufs=1, name=f"acc{g['t']}{g['p']}"
            )

        def emit_pv(g, u):
            p = g["p"]
            for j in range(2):
                e, off = unit_map(u, j)
                nc.tensor.matmul(
                    g["acc"][:, j * 512 : (j + 1) * 512],
                    vplus[u][:, 2 * p + j, :],
                    g["probs"][e][:, off : off + 512],
                    start=(u == 0),
                    stop=(u == NB - 1),
                )

        def normalize(g):
            # attn = attn_unnorm / sumexp; sums live in psum row 64.  The
            # acc slot is released after two quick DVE copies; the recip /
            # broadcast / multiply run off-slot on SBUF data.
            t, p = g["t"], g["p"]
            tsl = slice(t * 512, (t + 1) * 512)
            ps_at = g["acc"]
            sums = smallp.tile([1, 1024], F32, tag="sums")
            nc.vector.tensor_copy(sums[:], ps_at[DK : DK + 1, :])
            attw = smallp.tile([64, 1024], BF16, tag="attw")
            nc.vector.tensor_copy(attw[:], ps_at[0:DK, :])
            recip = smallp.tile([1, 1024], F32, tag="recip")
            nc.vector.reciprocal_approx_fast(recip[:], sums[:])
            rec_b = smallp.tile([64, 1024], F32, tag="rec_b", bufs=1)
            tmp = smallp.tile([64, 512], BF16, tag="tmpn")
            for hh in range(2):
                hsl2 = slice(hh * 512, (hh + 1) * 512)
                nc.gpsimd.partition_broadcast(rec_b[:, hsl2], recip[0:1, hsl2])
                if hh == 0:
                    nc.vector.tensor_mul(attnT[p][0:64, tsl], attw[:, hsl2], rec_b[:, hsl2])
                else:
                    nc.vector.tensor_mul(tmp[:], attw[:, hsl2], rec_b[:, hsl2])
                    nc.sync.dma_start(out=attnT[p][64:128, tsl], in_=tmp[:])
            g["acc"] = None
            g["probs"] = [None] * NE

        def out_proj_block(sb, tag="acc"):
            ps_o = psum.tile(
                [128, 1024], F32, tag=tag, bufs=(1 if tag == "acc" else 2),
                name=f"po{sb}",
            )
            for n in range(2):
                for p2 in range(2):
                    nc.tensor.matmul(
                        ps_o[:, n * 512 : (n + 1) * 512],
                        attnT[p2][:, sb * 128 : (sb + 1) * 128],
                        wo_sb[:, p2, n * 512 : (n + 1) * 512],
                        start=(p2 == 0),
                        stop=(p2 == 1),
                    )
            o_sb = outp.tile([128, 1024], BF16, tag="osb")
            nc.vector.tensor_copy(o_sb[:], ps_o[:])
            nc.sync.dma_start(out=out_d[sb * 128 : (sb + 1) * 128, :], in_=o_sb[:])

        # ================= schedule =================
        # prologue projections (sc slots; before any scores exist).
        # scores unit u only needs kT cols [128u, 128u+128) = chunk u//4.
        proj_group(wk_sb, bk_sb, kT, k_h, 0, 0, "sc")
        proj_group(wq_sb, bq_sb, qT, q_h, 0, 0, "sc")

        # work queue: (cost_ns, gate_unit, fn).  Strict FIFO; an item is
        # emitted only once the global unit index reaches its gate (keeps
        # the in-order PE stream from blocking ahead of pending scores)
        # and its cost fits the per-unit slack budget.
        work = []

        def W(cost, gate, fn):
            work.append((cost, gate, fn))

        # filler projections + vproj (acc slot; gate-monotone FIFO).
        for s in range(1, NS):
            W(1200, s - 1, lambda s=s: proj_group(wk_sb, bk_sb, kT, k_h, 0, s, "acc"))
        for s in range(NS):
            W(1200, 3 + s, lambda s=s: proj_group(wk_sb, bk_sb, kT, k_h, 1, s, "acc"))
        W(1200, 7, lambda: proj_group(wq_sb, bq_sb, qT, q_h, 1, 0, "acc"))
        for sb in range(8):
            W(900, 9 + sb, lambda sb=sb: vproj_one(sb))
        for sb in range(8, NB):
            W(900, 11 + sb, lambda sb=sb: vproj_one(sb))
        W(1200, 27, lambda: proj_group(wq_sb, bq_sb, qT, q_h, 0, 1, "acc"))
        W(1200, 28, lambda: proj_group(wq_sb, bq_sb, qT, q_h, 1, 1, "acc"))

        groups = [
            {"t": t, "p": p, "sc": [None] * NE, "probs": [None] * NE, "acc": None}
            for t in range(NS)
            for p in range(2)
        ]

        state = {"unit": 0, "budget": 0.0}

        def pump():
            while work:
                cost, gate, fn = work[0]
                if gate > state["unit"] or cost > state["budget"]:
                    return
                work.pop(0)
                fn()
                state["budget"] -= cost

        UNIT_SLACK = 950.0  # ns of queue work per scores unit

        for gi, g in enumerate(groups):
            pend = []
            for u in range(NB):
                pend += emit_scores(g, u)
                while pend:
                    emit_exp(g, pend.pop(0))
                state["unit"] += 1
                state["budget"] = min(state["budget"] + UNIT_SLACK, 4 * UNIT_SLACK)
                pump()
            # enqueue this group's pv chain + normalize (+ out-proj when a
            # tile completes).  pv(g, u) gated on its exp's emission unit.
            base = 16 * gi

            def mk_acc(g=g):
                alloc_acc(g)

            W(0, 0, mk_acc)
            for u in range(NB):
                W(
                    450,
                    base + exp_done_unit(unit_map(u, 1)[0]) + 2,
                    lambda g=g, u=u: emit_pv(g, u),
                )
            # normalize cost models the DVE latency the acc slot stays held
            W(1000, 0, lambda g=g: normalize(g))
            if gi == 1:
                for s in (2, 3):
                    for m in range(2):
                        W(1200, 0, lambda m=m, s=s: proj_group(wq_sb, bq_sb, qT, q_h, m, s, "acc"))
            if g["p"] == 1:
                t = g["t"]
                tag = "sc" if t == NS - 1 else "acc"
                for bi in range(4):
                    W(1400, 0, lambda sb=4 * t + bi, tag=tag: out_proj_block(sb, tag))

        # tail: drain the queue (Tile sems own correctness; the last
        # out-proj blocks pipeline through the freed sc slots)
        while work:
            cost, gate, fn = work.pop(0)
            fn()

    nc.finalize()
    return nc


def kernel(Q, K, V, Wq, bq, Wk, bk, Wv, bv, Wo, bo):
    from concourse.bass_utils import run_bass_kernel_spmd

    Q, K, V = (np.asarray(a, dtype=np.float32) for a in (Q, K, V))
    Wq, bq, Wk, bk = (np.asarray(a, dtype=np.float32) for a in (Wq, bq, Wk, bk))
    Wv, bv, Wo, bo = (np.asarray(a, dtype=np.float32) for a in (Wv, bv, Wo, bo))

    if "nc" not in _CACHE:
        _CACHE["nc"] = _build_nc()
    nc = _CACHE["nc"]

    import ml_dtypes

    bf16 = ml_dtypes.bfloat16
    # fold log2(e) * 1/sqrt(dk) into the q projection: scores emerge in
    # log2 domain and Exp(scale=ln2) turns them into 2^t
    lam = np.float32(LOG2E * SCALE)
    Wq_s = Wq * lam
    bq_s = bq * lam
    qts = [np.ascontiguousarray(Q[b].T).astype(bf16) for b in range(B)]
    kts = [np.ascontiguousarray(K[b].T).astype(bf16) for b in range(B)]
    vts = [np.ascontiguousarray(V[b].T).astype(bf16) for b in range(B)]
    in_maps = []
    for c in range(NCORES):
        b, g = divmod(c, 4)
        sl = slice(g * HD, (g + 1) * HD)
        in_maps.append(
            {
                "qt": qts[b],
                "kt": kts[b],
                "vt": vts[b],
                "wqt": np.ascontiguousarray(Wq_s[sl, :].T).astype(bf16),
                "wkt": np.ascontiguousarray(Wk[sl, :].T).astype(bf16),
                "wvt": np.ascontiguousarray(Wv[sl, :].T).astype(bf16),
                "wot": np.ascontiguousarray(Wo[:, sl].T).astype(bf16),
                "bq": np.ascontiguousarray(bq_s[sl]),
                "bk": np.ascontiguousarray(bk[sl]),
            }
        )

    res = run_bass_kernel_spmd(nc, in_maps, core_ids=list(range(NCORES)))

    out = np.zeros((B, S, D), dtype=np.float32)
    for c in range(NCORES):
        out[c // 4] += res.results[c]["out"].astype(np.float32)
    # bo_eff = bo + Wo @ bv  (value bias commutes through the normalized
    # attention since each probability row sums to 1)
    out += bo + Wo @ bv
    return out


# revision 17
# speedup vs baseline: 1.0023x; 1.0023x over previous
"""Multi-head attention (B=2, S=2048, D=1024, H=16) on 8 trn2 NeuronCores.

Sharding: core c = (b, g) with b = c // 4 (data parallel over batch) and
g = c % 4 (tensor parallel over heads, 4 heads per core).  Each core
computes q/k/v projections for its 4 heads, attention, and a partial
output projection (row-parallel Wo); the host sums the 4 partials per
batch and adds bo_eff = bo + Wo @ bv.

v3 schedule: the ACT engine (exp over the 16.8M scores per core) is the
hard bottleneck (~0.85ns/col + ~450ns per-ACTIVATE overhead), so the
kernel keeps ACT busy continuously from the earliest possible moment:

- exp instructions are N=1536 (3 PSUM banks x 2 buffers; 10x1536 + 1024
  per (sq-tile, head-pair) group), amortizing the ACTIVATE overhead.
- only K-proj(m0) + Q-proj(m0,s0) precede the first scores; every other
  projection (k m1, q s1-3, v) plus PV / normalize / out-proj is fed
  from a budgeted work queue pumped in the PE slack of each scores unit,
  so the in-order PE stream never blocks ahead of pending scores.
- inputs stage as single [128, 8, 2048] SBUF tiles (one DMA issue per
  512-col chunk) split over the sync/vector/gpsimd queues.
- PE is pre-warmed with dummy matmuls during the DMA head (HAM at full
  clock when the projections start).
- PSUM: 2x3-bank rotating scores slots + one 2-bank slot shared (in
  strict FIFO) by warmup, filler projections, v-proj, PV accumulation
  and interleaved out-projection.
- softmax denominators ride as a ones-column in v (row 64 of the PV
  accumulator); normalization divides attn rows; the reciprocal reads
  the sums row directly from PSUM; all PSUM->SBUF copies are on DVE.
"""

import contextlib
import sys

import numpy as np

for _p in ("/opt/trn_rl_repo", "/root/.axon_site/_ro/trn_rl_repo"):
    if _p not in sys.path:
        sys.path.insert(0, _p)

B, S, D = 2, 2048, 1024
H, DK = 16, 64
HPC = 4  # heads per core
HD = HPC * DK  # 256 head-dims per core
NCORES = 8
SCALE = 1.0 / 8.0  # 1/sqrt(DK)
LOG2E = float(np.log2(np.e))
LN2 = float(np.log(2.0))

KT = D // 128  # 8 contraction tiles for the projections
NS = S // 512  # 4 sq tiles
NB = S // 128  # 16 sk blocks (units) per group
EXPW = 1536  # exp instruction width (3 psum banks)
NE = 11  # exps per group: 10 x 1536 + 1 x 1024

_CACHE = {}


def _build_nc():
    from concourse import bacc
    import concourse.mybir as mybir
    import concourse.tile as tile

    F32 = mybir.dt.float32
    BF16 = mybir.dt.bfloat16
    Exp = mybir.ActivationFunctionType.Exp

    nc = bacc.Bacc(None)

    qt_d = nc.dram_tensor("qt", [D, S], BF16, kind="ExternalInput")
    kt_d = nc.dram_tensor("kt", [D, S], BF16, kind="ExternalInput")
    vt_d = nc.dram_tensor("vt", [D, S], BF16, kind="ExternalInput")
    w1_d = nc.dram_tensor("w1", [128, 4096], BF16, kind="ExternalInput")  # wk|wq
    w2_d = nc.dram_tensor("w2", [128, 4096], BF16, kind="ExternalInput")  # wv|wo
    bias_d = nc.dram_tensor("bias", [128, 4], F32, kind="ExternalInput")  # bq|bk
    out_d = nc.dram_tensor("out", [S, D], BF16, kind="ExternalOutput")

    with tile.TileContext(nc) as tc, contextlib.ExitStack() as ctx:
        consts = ctx.enter_context(tc.tile_pool(name="consts", bufs=1))
        wpool = ctx.enter_context(tc.tile_pool(name="wpool", bufs=2))
        xpool = ctx.enter_context(tc.tile_pool(name="xpool", bufs=5))
        persist = ctx.enter_context(tc.tile_pool(name="persist", bufs=1))
        probsp = ctx.enter_context(tc.tile_pool(name="probsp", bufs=20))
        smallp = ctx.enter_context(tc.tile_pool(name="smallp", bufs=1))
        outp = ctx.enter_context(tc.tile_pool(name="outp", bufs=2))
        psum = ctx.enter_context(tc.tile_pool(name="psum", bufs=2, space="PSUM"))

        # ---- weights/biases: host-packed so every DMA moves contiguous
        # multi-KB rows (tiny descriptors starve the DGE) ----
        bias_sb = consts.tile([128, 4], F32)
        nc.sync.dma_start(out=bias_sb[:], in_=bias_d[:])
        bq_sb = bias_sb[:, 0:2]  # col m = bq[128m : 128(m+1)]
        bk_sb = bias_sb[:, 2:4]

        w1_sb = wpool.tile([128, 4096], BF16, tag="w", name="w1")
        nc.sync.dma_start(out=w1_sb[:], in_=w1_d[:])
        wk_sb = w1_sb[:, 0:2048].rearrange("p (kt m) -> p kt m", kt=KT)
        wq_sb = w1_sb[:, 2048:4096].rearrange("p (kt m) -> p kt m", kt=KT)
        w2_sb = wpool.tile([128, 4096], BF16, tag="w", name="w2")
        nc.gpsimd.dma_start(out=w2_sb[:], in_=w2_d[:])
        wv_sb = w2_sb[:, 0:2048].rearrange("p (kt m) -> p kt m", kt=KT)
        wo_sb = w2_sb[:, 2048:4096].rearrange("p (m n) -> p m n", m=2)

        ones_sb = consts.tile([128, HPC], F32)
        nc.vector.memset(ones_sb[:], 1.0)
        # load the ACT exp table immediately so the first real exp is fast
        warm = consts.tile([1, 2], F32)
        nc.scalar.activation(warm[:], ones_sb[0:1, 0:2], Exp, scale=LN2)

        # ---- input staging: half-S tiles [128, KT, 1024].  K paces the
        # score stream (sync ring); Q s0 + V h0 land early on the scalar
        # ring; V h1 reuses K h0's slot, so its WAR-gated issue rides the
        # sync ring where nothing urgent queues behind it. ----
        def halfload(eng, dram, dst, h):
            eng.dma_start(
                out=dst[:, :, :],
                in_=dram[:, h * 1024 : (h + 1) * 1024].rearrange(
                    "(kt p) c -> p kt c", p=128
                ),
            )

        k_h = [xpool.tile([128, KT, 1024], BF16, tag="x", name=f"k_h{h}") for h in range(2)]
        for h in range(2):
            halfload(nc.sync, kt_d, k_h[h], h)
        q_h0 = xpool.tile([128, KT, 1024], BF16, tag="x", name="q_h0")
        halfload(nc.scalar, qt_d, q_h0, 0)
        v_h0 = xpool.tile([128, KT, 1024], BF16, tag="x", name="v_h0")
        halfload(nc.scalar, vt_d, v_h0, 0)
        q_h1 = xpool.tile([128, KT, 1024], BF16, tag="x", name="q_h1")
        halfload(nc.scalar, qt_d, q_h1, 1)
        v_h1 = xpool.tile([128, KT, 1024], BF16, tag="x", name="v_h1")
        halfload(nc.sync, vt_d, v_h1, 1)
        q_h = [q_h0, q_h1]
        v_h = [v_h0, v_h1]

        # ---- persistent activations ----
        qT = [persist.tile([128, S], BF16, name=f"qT{m}") for m in range(2)]
        kT = [persist.tile([128, S], BF16, name=f"kT{m}") for m in range(2)]
        vplus = [
            persist.tile([128, HPC, DK + 1], BF16, name=f"vp{i}") for i in range(NB)
        ]
        for i in range(NB):
            nc.vector.tensor_copy(
                vplus[i][:, :, DK : DK + 1],
                ones_sb[:].rearrange("p (h o) -> p h o", o=1),
            )
        attnT = [persist.tile([128, S], BF16, name=f"attnT{p}") for p in range(2)]

        # ---- helpers ----
        def proj_group(w_sb, bias_sb, dst, x_halves, m, s, tag):
            ps = psum.tile(
                [128, 512], F32, tag=tag, bufs=(2 if tag == "sc" else 1),
                name=f"pj{m}{s}",
            )
            xh = x_halves[s // 2]
            col = (s % 2) * 512
            for k in range(KT):
                nc.tensor.matmul(
                    ps[:],
                    w_sb[:, k, m * 128 : (m + 1) * 128],
                    xh[:, k, col : col + 512],
                    start=(k == 0),
                    stop=(k == KT - 1),
                )
            nc.vector.tensor_scalar_add(
                dst[m][:, s * 512 : (s + 1) * 512], ps[:], bias_sb[:, m : m + 1]
            )

        vstate = {"ps": None}

        def vproj_one(sb):
            # all 16 blocks ping-pong through one 2-bank psum tile
            if vstate["ps"] is None:
                vstate["ps"] = psum.tile(
                    [128, 1024], F32, tag="acc", bufs=1, name="vprojps"
                )
            ps = vstate["ps"]
            col = (sb % 2) * 512
            vh = v_h[sb // 8]
            vcol = (sb % 8) * 128
            for k in range(KT):
                nc.tensor.matmul(
                    ps[:, col : col + HD],
                    vh[:, k, vcol : vcol + 128],
                    wv_sb[:, k, :],
                    start=(k == 0),
                    stop=(k == KT - 1),
                )
            nc.vector.tensor_copy(
                vplus[sb][:, :, 0:DK],
                ps[:, col : col + HD].rearrange("p (h d) -> p h d", h=HPC),
            )
            if sb == NB - 1:
                vstate["ps"] = None  # release slot for the pv chain

        def unit_map(u, j):
            # (unit, head) -> (exp index, col offset) within a group
            c = 1024 * u + 512 * j
            if c >= 15360:
                return 10, c - 15360
            return c // 1536, c % 1536

        def exp_done_unit(e):
            # unit at which exp e's last scores chunk is emitted
            if e == 10:
                return 15
            return (1536 * (e + 1) - 512) // 1024

        def emit_scores(g, u):
            t, p = g["t"], g["p"]
            tsl = slice(t * 512, (t + 1) * 512)
            done = []
            for j in range(2):
                e, off = unit_map(u, j)
                if g["sc"][e] is None:
                    w = EXPW if e < 10 else 1024
                    g["sc"][e] = psum.tile(
                        [128, w], F32, tag="sc", bufs=2, name=f"sc{t}{p}{e}"
                    )
                hsl = slice(64 * j, 64 * (j + 1))
                nc.tensor.matmul(
                    g["sc"][e][:, off : off + 512],
                    kT[p][hsl, u * 128 : (u + 1) * 128],
                    qT[p][hsl, tsl],
                    start=True,
                    stop=True,
                    tile_position=(64 * j, 0),
                )
                if (e < 10 and off == 1024) or (u, j) == (15, 1):
                    done.append(e)
            return done

        def emit_exp(g, e):
            w = EXPW if e < 10 else 1024
            probs = probsp.tile(
                [128, w], BF16, tag="probs", bufs=20, name=f"pr{g['t']}{g['p']}{e}"
            )
            nc.scalar.activation(probs[:], g["sc"][e][:], Exp, scale=LN2)
            g["probs"][e] = probs
            g["sc"][e] = None

        def alloc_acc(g):
            g["acc"] = psum.tile(
                [DK + 1, 1024], F32, tag="acc", bufs=1, name=f"acc{g['t']}{g['p']}"
            )

        def emit_pv(g, u):
            p = g["p"]
            for j in range(2):
                e, off = unit_map(u, j)
                nc.tensor.matmul(
                    g["acc"][:, j * 512 : (j + 1) * 512],
                    vplus[u][:, 2 * p + j, :],
                    g["probs"][e][:, off : off + 512],
                    start=(u == 0),
                    stop=(u == NB - 1),
                )

        def normalize(g):
            # attn = attn_unnorm / sumexp; sums live in psum row 64.  The
            # acc slot is released after two quick DVE copies; the recip /
            # broadcast / multiply run off-slot on SBUF data.
            t, p = g["t"], g["p"]
            tsl = slice(t * 512, (t + 1) * 512)
            ps_at = g["acc"]
            sums = smallp.tile([1, 1024], F32, tag="sums")
            nc.vector.tensor_copy(sums[:], ps_at[DK : DK + 1, :])
            attw = smallp.tile([64, 1024], BF16, tag="attw")
            nc.vector.tensor_copy(attw[:], ps_at[0:DK, :])
            recip = smallp.tile([1, 1024], F32, tag="recip")
            nc.vector.reciprocal_approx_fast(recip[:], sums[:])
            rec_b = smallp.tile([64, 1024], F32, tag="rec_b", bufs=1)
            tmp = smallp.tile([64, 512], BF16, tag="tmpn")
            for hh in range(2):
                hsl2 = slice(hh * 512, (hh + 1) * 512)
                nc.gpsimd.partition_broadcast(rec_b[:, hsl2], recip[0:1, hsl2])
                if hh == 0:
                    nc.vector.tensor_mul(attnT[p][0:64, tsl], attw[:, hsl2], rec_b[:, hsl2])
                else:
                    nc.vector.tensor_mul(tmp[:], attw[:, hsl2], rec_b[:, hsl2])
                    nc.sync.dma_start(out=attnT[p][64:128, tsl], in_=tmp[:])
            g["acc"] = None
            g["probs"] = [None] * NE

        def out_proj_block(sb, tag="acc"):
            ps_o = psum.tile(
                [128, 1024], F32, tag=tag, bufs=(1 if tag == "acc" else 2),
                name=f"po{sb}",
            )
            for n in range(2):
                for p2 in range(2):
                    nc.tensor.matmul(
                        ps_o[:, n * 512 : (n + 1) * 512],
                        attnT[p2][:, sb * 128 : (sb + 1) * 128],
                        wo_sb[:, p2, n * 512 : (n + 1) * 512],
                        start=(p2 == 0),
                        stop=(p2 == 1),
                    )
            o_sb = outp.tile([128, 1024], BF16, tag="osb")
            nc.vector.tensor_copy(o_sb[:], ps_o[:])
            nc.sync.dma_start(out=out_d[sb * 128 : (sb + 1) * 128, :], in_=o_sb[:])

        # ================= schedule =================
        # prologue projections (sc slots; before any scores exist).
        # scores unit u only needs kT cols [128u, 128u+128) = chunk u//4.
        proj_group(wk_sb, bk_sb, kT, k_h, 0, 0, "sc")
        proj_group(wq_sb, bq_sb, qT, q_h, 0, 0, "sc")

        # work queue: (cost_ns, gate_unit, fn).  Strict FIFO; an item is
        # emitted only once the global unit index reaches its gate (keeps
        # the in-order PE stream from blocking ahead of pending scores)
        # and its cost fits the per-unit slack budget.
        work = []

        def W(cost, gate, fn):
            work.append((cost, gate, fn))

        # filler projections + vproj (acc slot; gate-monotone FIFO).
        for s in range(1, NS):
            W(1200, s - 1, lambda s=s: proj_group(wk_sb, bk_sb, kT, k_h, 0, s, "acc"))
        for s in range(NS):
            W(1200, 3 + s, lambda s=s: proj_group(wk_sb, bk_sb, kT, k_h, 1, s, "acc"))
        W(1200, 7, lambda: proj_group(wq_sb, bq_sb, qT, q_h, 1, 0, "acc"))
        for sb in range(8):
            W(900, 9 + sb, lambda sb=sb: vproj_one(sb))
        for sb in range(8, NB):
            W(900, 11 + sb, lambda sb=sb: vproj_one(sb))
        W(1200, 27, lambda: proj_group(wq_sb, bq_sb, qT, q_h, 0, 1, "acc"))
        W(1200, 28, lambda: proj_group(wq_sb, bq_sb, qT, q_h, 1, 1, "acc"))

        groups = [
            {"t": t, "p": p, "sc": [None] * NE, "probs": [None] * NE, "acc": None}
            for t in range(NS)
            for p in range(2)
        ]

        state = {"unit": 0, "budget": 0.0}

        def pump():
            while work:
                cost, gate, fn = work[0]
                if gate > state["unit"] or cost > state["budget"]:
                    return
                work.pop(0)
                fn()
                state["budget"] -= cost

        UNIT_SLACK = 950.0  # ns of queue work per scores unit

        for gi, g in enumerate(groups):
            pend = []
            for u in range(NB):
                pend += emit_scores(g, u)
                while pend:
                    emit_exp(g, pend.pop(0))
                state["unit"] += 1
                state["budget"] = min(state["budget"] + UNIT_SLACK, 4 * UNIT_SLACK)
                pump()
            # enqueue this group's pv chain + normalize (+ out-proj when a
            # tile completes).  pv(g, u) gated on its exp's emission unit.
            base = 16 * gi

            def mk_acc(g=g):
                alloc_acc(g)

            W(0, 0, mk_acc)
            for u in range(NB):
                W(
                    450,
                    base + exp_done_unit(unit_map(u, 1)[0]) + 2,
                    lambda g=g, u=u: emit_pv(g, u),
                )
            # normalize cost models the DVE latency the acc slot stays held
            W(1000, 0, lambda g=g: normalize(g))
            if gi == 1:
                for s in (2, 3):
                    for m in range(2):
                        W(1200, 0, lambda m=m, s=s: proj_group(wq_sb, bq_sb, qT, q_h, m, s, "acc"))
            if g["p"] == 1:
                t = g["t"]
                tag = "sc" if t == NS - 1 else "acc"
                for bi in range(4):
                    W(1400, 0, lambda sb=4 * t + bi, tag=tag: out_proj_block(sb, tag))

        # tail: drain the queue (Tile sems own correctness; the last
        # out-proj blocks pipeline through the freed sc slots)
        while work:
            cost, gate, fn = work.pop(0)
            fn()

    nc.finalize()
    return nc


def kernel(Q, K, V, Wq, bq, Wk, bk, Wv, bv, Wo, bo):
    from concourse.bass_utils import run_bass_kernel_spmd

    Q, K, V = (np.asarray(a, dtype=np.float32) for a in (Q, K, V))
    Wq, bq, Wk, bk = (np.asarray(a, dtype=np.float32) for a in (Wq, bq, Wk, bk))
    Wv, bv, Wo, bo = (np.asarray(a, dtype=np.float32) for a in (Wv, bv, Wo, bo))

    if "nc" not in _CACHE:
        _CACHE["nc"] = _build_nc()
    nc = _CACHE["nc"]

    import ml_dtypes

    bf16 = ml_dtypes.bfloat16
    # fold log2(e) * 1/sqrt(dk) into the q projection: scores emerge in
    # log2 domain and Exp(scale=ln2) turns them into 2^t
    lam = np.float32(LOG2E * SCALE)
    Wq_s = Wq * lam
    bq_s = bq * lam
    qts = [np.ascontiguousarray(Q[b].T).astype(bf16) for b in range(B)]
    kts = [np.ascontiguousarray(K[b].T).astype(bf16) for b in range(B)]
    vts = [np.ascontiguousarray(V[b].T).astype(bf16) for b in range(B)]

    def pack_proj(w):  # [HD, D] slice -> [128, 2048] in [p][kt][m] layout
        wt = np.ascontiguousarray(w.T).astype(bf16)  # [D, HD]
        return wt.reshape(KT, 128, HD).transpose(1, 0, 2).reshape(128, KT * HD)

    def pack_wo(w):  # Wo[:, sl] -> [128, 2048] in [p][pair][dmodel] layout
        wt = np.ascontiguousarray(w.T).astype(bf16)  # [HD, D]
        return wt.reshape(2, 128, D).transpose(1, 0, 2).reshape(128, 2 * D)

    def pack_bias(b):  # [HD] -> [128, 2] col m = b[128m+p]
        return np.ascontiguousarray(b.reshape(2, 128).T)

    in_maps = []
    for c in range(NCORES):
        b, g = divmod(c, 4)
        sl = slice(g * HD, (g + 1) * HD)
        w1 = np.concatenate([pack_proj(Wk[sl, :]), pack_proj(Wq_s[sl, :])], axis=1)
        w2 = np.concatenate([pack_proj(Wv[sl, :]), pack_wo(Wo[:, sl])], axis=1)
        bias = np.concatenate([pack_bias(bq_s[sl]), pack_bias(bk[sl])], axis=1)
        in_maps.append(
            {
                "qt": qts[b],
                "kt": kts[b],
                "vt": vts[b],
                "w1": np.ascontiguousarray(w1),
                "w2": np.ascontiguousarray(w2),
                "bias": np.ascontiguousarray(bias.astype(np.float32)),
            }
        )

    res = run_bass_kernel_spmd(nc, in_maps, core_ids=list(range(NCORES)))

    out = np.zeros((B, S, D), dtype=np.float32)
    for c in range(NCORES):
        out[c // 4] += res.results[c]["out"].astype(np.float32)
    # bo_eff = bo + Wo @ bv  (value bias commutes through the normalized
    # attention since each probability row sums to 1)
    out += bo + Wo @ bv
    return out


# revision 19
# speedup vs baseline: 1.0555x; 1.0531x over previous
"""Multi-head attention (B=2, S=2048, D=1024, H=16) on 8 trn2 NeuronCores.

Sharding: core c = (b, g) with b = c // 4 (data parallel over batch) and
g = c % 4 (tensor parallel over heads, 4 heads per core).  Each core
computes q/k/v projections for its 4 heads, attention, and a partial
output projection (row-parallel Wo); the host sums the 4 partials per
batch and adds bo_eff = bo + Wo @ bv.

v3 schedule: the ACT engine (exp over the 16.8M scores per core) is the
hard bottleneck (~0.85ns/col + ~450ns per-ACTIVATE overhead), so the
kernel keeps ACT busy continuously from the earliest possible moment:

- exp instructions are N=1536 (3 PSUM banks x 2 buffers; 10x1536 + 1024
  per (sq-tile, head-pair) group), amortizing the ACTIVATE overhead.
- only K-proj(m0) + Q-proj(m0,s0) precede the first scores; every other
  projection (k m1, q s1-3, v) plus PV / normalize / out-proj is fed
  from a budgeted work queue pumped in the PE slack of each scores unit,
  so the in-order PE stream never blocks ahead of pending scores.
- inputs stage as single [128, 8, 2048] SBUF tiles (one DMA issue per
  512-col chunk) split over the sync/vector/gpsimd queues.
- PE is pre-warmed with dummy matmuls during the DMA head (HAM at full
  clock when the projections start).
- PSUM: 2x3-bank rotating scores slots + one 2-bank slot shared (in
  strict FIFO) by warmup, filler projections, v-proj, PV accumulation
  and interleaved out-projection.
- softmax denominators ride as a ones-column in v (row 64 of the PV
  accumulator); normalization divides attn rows; the reciprocal reads
  the sums row directly from PSUM; all PSUM->SBUF copies are on DVE.
"""

import contextlib
import sys

import numpy as np

for _p in ("/opt/trn_rl_repo", "/root/.axon_site/_ro/trn_rl_repo"):
    if _p not in sys.path:
        sys.path.insert(0, _p)

B, S, D = 2, 2048, 1024
H, DK = 16, 64
HPC = 4  # heads per core
HD = HPC * DK  # 256 head-dims per core
NCORES = 8
SCALE = 1.0 / 8.0  # 1/sqrt(DK)
LOG2E = float(np.log2(np.e))
LN2 = float(np.log(2.0))

KT = D // 128  # 8 contraction tiles for the projections
NS = S // 512  # 4 sq tiles
NB = S // 128  # 16 sk blocks (units) per group
EXPW = 1536  # exp instruction width (3 psum banks)
NE = 11  # exps per group: 10 x 1536 + 1 x 1024

_CACHE = {}


def _build_nc():
    from concourse import bacc
    import concourse.mybir as mybir
    import concourse.tile as tile

    F32 = mybir.dt.float32
    BF16 = mybir.dt.bfloat16
    Exp = mybir.ActivationFunctionType.Exp

    nc = bacc.Bacc(None)

    qt_d = nc.dram_tensor("qt", [D, S], BF16, kind="ExternalInput")
    kt_d = nc.dram_tensor("kt", [D, S], BF16, kind="ExternalInput")
    vt_d = nc.dram_tensor("vt", [D, S], BF16, kind="ExternalInput")
    w1_d = nc.dram_tensor("w1", [128, 4096], BF16, kind="ExternalInput")  # wk|wq
    w2_d = nc.dram_tensor("w2", [128, 4096], BF16, kind="ExternalInput")  # wv|wo
    bias_d = nc.dram_tensor("bias", [128, 4], F32, kind="ExternalInput")  # bq|bk
    out_d = nc.dram_tensor("out", [S, D], BF16, kind="ExternalOutput")

    with tile.TileContext(nc) as tc, contextlib.ExitStack() as ctx:
        consts = ctx.enter_context(tc.tile_pool(name="consts", bufs=1))
        wpool = ctx.enter_context(tc.tile_pool(name="wpool", bufs=2))
        xpool = ctx.enter_context(tc.tile_pool(name="xpool", bufs=5))
        persist = ctx.enter_context(tc.tile_pool(name="persist", bufs=1))
        probsp = ctx.enter_context(tc.tile_pool(name="probsp", bufs=20))
        smallp = ctx.enter_context(tc.tile_pool(name="smallp", bufs=1))
        outp = ctx.enter_context(tc.tile_pool(name="outp", bufs=2))
        psum = ctx.enter_context(tc.tile_pool(name="psum", bufs=2, space="PSUM"))

        # ---- weights/biases: host-packed so every DMA moves contiguous
        # multi-KB rows (tiny descriptors starve the DGE) ----
        bias_sb = consts.tile([128, 4], F32)
        nc.sync.dma_start(out=bias_sb[:], in_=bias_d[:])
        bq_sb = bias_sb[:, 0:2]  # col m = bq[128m : 128(m+1)]
        bk_sb = bias_sb[:, 2:4]

        w1_sb = wpool.tile([128, 4096], BF16, tag="w", name="w1")
        nc.sync.dma_start(out=w1_sb[:], in_=w1_d[:])
        wk_sb = w1_sb[:, 0:2048].rearrange("p (kt m) -> p kt m", kt=KT)
        wq_sb = w1_sb[:, 2048:4096].rearrange("p (kt m) -> p kt m", kt=KT)
        w2_sb = wpool.tile([128, 4096], BF16, tag="w", name="w2")
        nc.gpsimd.dma_start(out=w2_sb[:], in_=w2_d[:])
        wv_sb = w2_sb[:, 0:2048].rearrange("p (kt m) -> p kt m", kt=KT)
        wo_sb = w2_sb[:, 2048:4096].rearrange("p (m n) -> p m n", m=2)

        ones_sb = consts.tile([128, HPC], F32)
        nc.vector.memset(ones_sb[:], 1.0)
        # load the ACT exp table immediately so the first real exp is fast
        warm = consts.tile([1, 2], F32)
        nc.scalar.activation(warm[:], ones_sb[0:1, 0:2], Exp, scale=LN2)

        # ---- input staging: half-S tiles [128, KT, 1024].  K paces the
        # score stream (sync ring); Q s0 + V h0 land early on the scalar
        # ring; V h1 reuses K h0's slot, so its WAR-gated issue rides the
        # sync ring where nothing urgent queues behind it. ----
        def halfload(eng, dram, dst, h):
            eng.dma_start(
                out=dst[:, :, :],
                in_=dram[:, h * 1024 : (h + 1) * 1024].rearrange(
                    "(kt p) c -> p kt c", p=128
                ),
            )

        k_h = [xpool.tile([128, KT, 1024], BF16, tag="x", name=f"k_h{h}") for h in range(2)]
        halfload(nc.sync, kt_d, k_h[0], 0)
        q_h0 = xpool.tile([128, KT, 1024], BF16, tag="x", name="q_h0")
        halfload(nc.sync, qt_d, q_h0, 0)
        halfload(nc.sync, kt_d, k_h[1], 1)
        v_h0 = xpool.tile([128, KT, 1024], BF16, tag="x", name="v_h0")
        halfload(nc.sync, vt_d, v_h0, 0)
        q_h1 = xpool.tile([128, KT, 1024], BF16, tag="x", name="q_h1")
        halfload(nc.sync, qt_d, q_h1, 1)
        v_h1 = xpool.tile([128, KT, 1024], BF16, tag="x", name="v_h1")
        halfload(nc.sync, vt_d, v_h1, 1)
        q_h = [q_h0, q_h1]
        v_h = [v_h0, v_h1]

        # ---- persistent activations ----
        qT = [persist.tile([128, S], BF16, name=f"qT{m}") for m in range(2)]
        kT = [persist.tile([128, S], BF16, name=f"kT{m}") for m in range(2)]
        vplus = [
            persist.tile([128, HPC, DK + 1], BF16, name=f"vp{i}") for i in range(NB)
        ]
        for i in range(NB):
            nc.vector.tensor_copy(
                vplus[i][:, :, DK : DK + 1],
                ones_sb[:].rearrange("p (h o) -> p h o", o=1),
            )
        attnT = [persist.tile([128, S], BF16, name=f"attnT{p}") for p in range(2)]

        # ---- helpers ----
        def proj_group(w_sb, bias_sb, dst, x_halves, m, s, tag):
            ps = psum.tile(
                [128, 512], F32, tag=tag, bufs=(2 if tag == "sc" else 1),
                name=f"pj{m}{s}",
            )
            xh = x_halves[s // 2]
            col = (s % 2) * 512
            for k in range(KT):
                nc.tensor.matmul(
                    ps[:],
                    w_sb[:, k, m * 128 : (m + 1) * 128],
                    xh[:, k, col : col + 512],
                    start=(k == 0),
                    stop=(k == KT - 1),
                )
            nc.vector.tensor_scalar_add(
                dst[m][:, s * 512 : (s + 1) * 512], ps[:], bias_sb[:, m : m + 1]
            )

        vstate = {"ps": None}

        def vproj_one(sb):
            # all 16 blocks ping-pong through one 2-bank psum tile
            if vstate["ps"] is None:
                vstate["ps"] = psum.tile(
                    [128, 1024], F32, tag="acc", bufs=1, name="vprojps"
                )
            ps = vstate["ps"]
            col = (sb % 2) * 512
            vh = v_h[sb // 8]
            vcol = (sb % 8) * 128
            for k in range(KT):
                nc.tensor.matmul(
                    ps[:, col : col + HD],
                    vh[:, k, vcol : vcol + 128],
                    wv_sb[:, k, :],
                    start=(k == 0),
                    stop=(k == KT - 1),
                )
            nc.vector.tensor_copy(
                vplus[sb][:, :, 0:DK],
                ps[:, col : col + HD].rearrange("p (h d) -> p h d", h=HPC),
            )
            if sb == NB - 1:
                vstate["ps"] = None  # release slot for the pv chain

        def unit_map(u, j):
            # (unit, head) -> (exp index, col offset) within a group
            c = 1024 * u + 512 * j
            if c >= 15360:
                return 10, c - 15360
            return c // 1536, c % 1536

        def exp_done_unit(e):
            # unit at which exp e's last scores chunk is emitted
            if e == 10:
                return 15
            return (1536 * (e + 1) - 512) // 1024

        def emit_scores(g, u):
            t, p = g["t"], g["p"]
            tsl = slice(t * 512, (t + 1) * 512)
            done = []
            for j in range(2):
                e, off = unit_map(u, j)
                if g["sc"][e] is None:
                    w = EXPW if e < 10 else 1024
                    g["sc"][e] = psum.tile(
                        [128, w], F32, tag="sc", bufs=2, name=f"sc{t}{p}{e}"
                    )
                hsl = slice(64 * j, 64 * (j + 1))
                nc.tensor.matmul(
                    g["sc"][e][:, off : off + 512],
                    kT[p][hsl, u * 128 : (u + 1) * 128],
                    qT[p][hsl, tsl],
                    start=True,
                    stop=True,
                    tile_position=(64 * j, 0),
                )
                if (e < 10 and off == 1024) or (u, j) == (15, 1):
                    done.append(e)
            return done

        def emit_exp(g, e):
            w = EXPW if e < 10 else 1024
            probs = probsp.tile(
                [128, w], BF16, tag="probs", bufs=20, name=f"pr{g['t']}{g['p']}{e}"
            )
            nc.scalar.activation(probs[:], g["sc"][e][:], Exp, scale=LN2)
            g["probs"][e] = probs
            g["sc"][e] = None

        def alloc_acc(g):
            g["acc"] = psum.tile(
                [DK + 1, 1024], F32, tag="acc", bufs=1, name=f"acc{g['t']}{g['p']}"
            )

        def emit_pv(g, u):
            p = g["p"]
            for j in range(2):
                e, off = unit_map(u, j)
                nc.tensor.matmul(
                    g["acc"][:, j * 512 : (j + 1) * 512],
                    vplus[u][:, 2 * p + j, :],
                    g["probs"][e][:, off : off + 512],
                    start=(u == 0),
                    stop=(u == NB - 1),
                )

        def normalize(g):
            # attn = attn_unnorm / sumexp; sums live in psum row 64.  The
            # acc slot is released after two quick DVE copies; the recip /
            # broadcast / multiply run off-slot on SBUF data.
            t, p = g["t"], g["p"]
            tsl = slice(t * 512, (t + 1) * 512)
            ps_at = g["acc"]
            sums = smallp.tile([1, 1024], F32, tag="sums")
            nc.vector.tensor_copy(sums[:], ps_at[DK : DK + 1, :])
            attw = smallp.tile([64, 1024], BF16, tag="attw")
            nc.vector.tensor_copy(attw[:], ps_at[0:DK, :])
            recip = smallp.tile([1, 1024], F32, tag="recip")
            nc.vector.reciprocal_approx_fast(recip[:], sums[:])
            rec_b = smallp.tile([64, 1024], F32, tag="rec_b", bufs=1)
            tmp = smallp.tile([64, 512], BF16, tag="tmpn")
            for hh in range(2):
                hsl2 = slice(hh * 512, (hh + 1) * 512)
                nc.gpsimd.partition_broadcast(rec_b[:, hsl2], recip[0:1, hsl2])
                if hh == 0:
                    nc.vector.tensor_mul(attnT[p][0:64, tsl], attw[:, hsl2], rec_b[:, hsl2])
                else:
                    nc.vector.tensor_mul(tmp[:], attw[:, hsl2], rec_b[:, hsl2])
                    nc.sync.dma_start(out=attnT[p][64:128, tsl], in_=tmp[:])
            g["acc"] = None
            g["probs"] = [None] * NE

        def out_proj_block(sb, tag="acc"):
            ps_o = psum.tile(
                [128, 1024], F32, tag=tag, bufs=(1 if tag == "acc" else 2),
                name=f"po{sb}",
            )
            for n in range(2):
                for p2 in range(2):
                    nc.tensor.matmul(
                        ps_o[:, n * 512 : (n + 1) * 512],
                        attnT[p2][:, sb * 128 : (sb + 1) * 128],
                        wo_sb[:, p2, n * 512 : (n + 1) * 512],
                        start=(p2 == 0),
                        stop=(p2 == 1),
                    )
            o_sb = outp.tile([128, 1024], BF16, tag="osb")
            nc.vector.tensor_copy(o_sb[:], ps_o[:])
            nc.sync.dma_start(out=out_d[sb * 128 : (sb + 1) * 128, :], in_=o_sb[:])

        # ================= schedule =================
        # prologue projections (sc slots; before any scores exist).
        # scores unit u only needs kT cols [128u, 128u+128) = chunk u//4.
        proj_group(wk_sb, bk_sb, kT, k_h, 0, 0, "sc")
        proj_group(wq_sb, bq_sb, qT, q_h, 0, 0, "sc")

        # work queue: (cost_ns, gate_unit, fn).  Strict FIFO; an item is
        # emitted only once the global unit index reaches its gate (keeps
        # the in-order PE stream from blocking ahead of pending scores)
        # and its cost fits the per-unit slack budget.
        work = []

        def W(cost, gate, fn):
            work.append((cost, gate, fn))

        # filler projections + vproj (acc slot; gate-monotone FIFO).
        # costs are PE stream-time estimates; gates track DMA arrival.
        W(1700, 0, lambda: proj_group(wk_sb, bk_sb, kT, k_h, 0, 1, "acc"))
        W(1700, 4, lambda: proj_group(wk_sb, bk_sb, kT, k_h, 0, 2, "acc"))
        W(1700, 5, lambda: proj_group(wk_sb, bk_sb, kT, k_h, 0, 3, "acc"))
        for s in range(NS):
            W(1700, 6 + s, lambda s=s: proj_group(wk_sb, bk_sb, kT, k_h, 1, s, "acc"))
        W(1700, 10, lambda: proj_group(wq_sb, bq_sb, qT, q_h, 1, 0, "acc"))
        for sb in range(8):
            W(860, 11 + sb, lambda sb=sb: vproj_one(sb))
        for sb in range(8, NB):
            W(860, 13 + sb, lambda sb=sb: vproj_one(sb))
        W(1700, 29, lambda: proj_group(wq_sb, bq_sb, qT, q_h, 0, 1, "acc"))
        W(1700, 30, lambda: proj_group(wq_sb, bq_sb, qT, q_h, 1, 1, "acc"))

        groups = [
            {"t": t, "p": p, "sc": [None] * NE, "probs": [None] * NE, "acc": None}
            for t in range(NS)
            for p in range(2)
        ]

        state = {"unit": 0, "budget": 0.0}

        def pump():
            # items overdue by 3+ units are emitted regardless of budget —
            # a group's scores must never precede their filler producers in
            # the in-order PE stream (hard deadlock otherwise)
            while work:
                cost, gate, fn = work[0]
                overdue = gate <= state["unit"] - 3
                if not overdue and (gate > state["unit"] or cost > state["budget"]):
                    return
                work.pop(0)
                fn()
                state["budget"] = max(state["budget"] - cost, -4 * UNIT_SLACK)

        UNIT_SLACK = 830.0  # ns of queue work per scores unit

        for gi, g in enumerate(groups):
            pend = []
            for u in range(NB):
                pend += emit_scores(g, u)
                while pend:
                    emit_exp(g, pend.pop(0))
                state["unit"] += 1
                state["budget"] = min(state["budget"] + UNIT_SLACK, 4 * UNIT_SLACK)
                pump()
            # enqueue this group's pv chain + normalize (+ out-proj when a
            # tile completes).  pv(g, u) gated on its exp's emission unit.
            base = 16 * gi

            def mk_acc(g=g):
                alloc_acc(g)

            W(0, 0, mk_acc)
            for u in range(NB):
                W(
                    430,
                    base + exp_done_unit(unit_map(u, 1)[0]) + 2,
                    lambda g=g, u=u: emit_pv(g, u),
                )
            # normalize cost models the DVE latency the acc slot stays held
            W(500, 0, lambda g=g: normalize(g))
            if gi == 1:
                for s in (2, 3):
                    for m in range(2):
                        W(1700, 0, lambda m=m, s=s: proj_group(wq_sb, bq_sb, qT, q_h, m, s, "acc"))
            if g["p"] == 1:
                t = g["t"]
                tag = "sc" if t == NS - 1 else "acc"
                for bi in range(4):
                    W(900, 0, lambda sb=4 * t + bi, tag=tag: out_proj_block(sb, tag))

        # tail: drain the queue (Tile sems own correctness; the last
        # out-proj blocks pipeline through the freed sc slots)
        while work:
            cost, gate, fn = work.pop(0)
            fn()

    nc.finalize()
    return nc


def kernel(Q, K, V, Wq, bq, Wk, bk, Wv, bv, Wo, bo):
    from concourse.bass_utils import run_bass_kernel_spmd

    Q, K, V = (np.asarray(a, dtype=np.float32) for a in (Q, K, V))
    Wq, bq, Wk, bk = (np.asarray(a, dtype=np.float32) for a in (Wq, bq, Wk, bk))
    Wv, bv, Wo, bo = (np.asarray(a, dtype=np.float32) for a in (Wv, bv, Wo, bo))

    if "nc" not in _CACHE:
        _CACHE["nc"] = _build_nc()
    nc = _CACHE["nc"]

    import ml_dtypes

    bf16 = ml_dtypes.bfloat16
    # fold log2(e) * 1/sqrt(dk) into the q projection: scores emerge in
    # log2 domain and Exp(scale=ln2) turns them into 2^t
    lam = np.float32(LOG2E * SCALE)
    Wq_s = Wq * lam
    bq_s = bq * lam
    qts = [np.ascontiguousarray(Q[b].T).astype(bf16) for b in range(B)]
    kts = [np.ascontiguousarray(K[b].T).astype(bf16) for b in range(B)]
    vts = [np.ascontiguousarray(V[b].T).astype(bf16) for b in range(B)]

    def pack_proj(w):  # [HD, D] slice -> [128, 2048] in [p][kt][m] layout
        wt = np.ascontiguousarray(w.T).astype(bf16)  # [D, HD]
        return wt.reshape(KT, 128, HD).transpose(1, 0, 2).reshape(128, KT * HD)

    def pack_wo(w):  # Wo[:, sl] -> [128, 2048] in [p][pair][dmodel] layout
        wt = np.ascontiguousarray(w.T).astype(bf16)  # [HD, D]
        return wt.reshape(2, 128, D).transpose(1, 0, 2).reshape(128, 2 * D)

    def pack_bias(b):  # [HD] -> [128, 2] col m = b[128m+p]
        return np.ascontiguousarray(b.reshape(2, 128).T)

    in_maps = []
    for c in range(NCORES):
        b, g = divmod(c, 4)
        sl = slice(g * HD, (g + 1) * HD)
        w1 = np.concatenate([pack_proj(Wk[sl, :]), pack_proj(Wq_s[sl, :])], axis=1)
        w2 = np.concatenate([pack_proj(Wv[sl, :]), pack_wo(Wo[:, sl])], axis=1)
        bias = np.concatenate([pack_bias(bq_s[sl]), pack_bias(bk[sl])], axis=1)
        in_maps.append(
            {
                "qt": qts[b],
                "kt": kts[b],
                "vt": vts[b],
                "w1": np.ascontiguousarray(w1),
                "w2": np.ascontiguousarray(w2),
                "bias": np.ascontiguousarray(bias.astype(np.float32)),
            }
        )

    res = run_bass_kernel_spmd(nc, in_maps, core_ids=list(range(NCORES)))

    out = np.zeros((B, S, D), dtype=np.float32)
    for c in range(NCORES):
        out[c // 4] += res.results[c]["out"].astype(np.float32)
    # bo_eff = bo + Wo @ bv  (value bias commutes through the normalized
    # attention since each probability row sums to 1)
    out += bo + Wo @ bv
    return out
